# revision 1
# baseline (speedup 1.0000x reference)
"""Fused bmm + residual kernel for Trainium2 (8 NeuronCores, data-parallel).

out[n,c,p] = x[n,c,p] + alpha * sum_q attn[n,p,q] * D[n,q,c]
  N=2048, C=512, H=W=7 (HW=49)

Sharding: batch N across 8 cores (256 each). Each core computes its slice
independently; no collectives.

Per-core scheme (channel-interleaved, pair-packed):
 - SBUF x/out tiles [128, G, 196]: partition r holds channels {4r..4r+3}
   -> 784B-contiguous DMA runs at full 128 partitions.
 - D tiles [128, G/2, 512] in "gap" layout: partition b*64+q holds
   D[pair_batch b, q, :]; rows 49:64 and 113:128 are zeroed once.
 - attn transposed on PE: in [49, 2x64-slot pair] -> out [128, 49] with
   A^T(even) at rows 0:49, A^T(odd) at rows 64:113.
 - rhs [128, 2, 49]: alpha*A^T(even) at rows 0:49 col-block 0,
   alpha*A^T(odd) at rows 64:113 col-block 1, zeros elsewhere.
 - 4 matmuls per pair (chunk j: channels c==j mod 4), K=113, M=128, N=98;
   zero rhs rows annihilate the cross-batch terms.
 - residual add on DVE straight from PSUM, store via ACT-ring DMA.
"""
import sys

sys.path.insert(0, "/opt/trn_rl_repo")

import numpy as np

# ---- static problem config (hardcoded per harness contract) ----
N_TOT, C, HW = 2048, 512, 49
N_CORES = 8
NB = N_TOT // N_CORES        # 256 batches per core
G = 16                       # batches per group (one DMA round)
NPAIR = G // 2               # pairs per group
NGROUP = NB // G             # groups per core
NBD = 4                      # rhs ring size
ND = 3                       # D-tile ring size

_cached = {}


def _build_bass():
    import concourse.bacc as bacc
    import concourse.mybir as mybir
    from concourse import tile

    f32 = mybir.dt.float32
    nc = bacc.Bacc(None, target_bir_lowering=False)

    x_d = nc.dram_tensor("x", [NB, C, HW], f32, kind="ExternalInput")
    a_d = nc.dram_tensor("attn", [NB, HW, HW], f32, kind="ExternalInput")
    d_d = nc.dram_tensor("d", [NB, HW, C], f32, kind="ExternalInput")
    al_d = nc.dram_tensor("alphac", [128, 1], f32, kind="ExternalInput")
    id_d = nc.dram_tensor("ident", [HW, HW], f32, kind="ExternalInput")
    o_d = nc.dram_tensor("out", [NB, C, HW], f32, kind="ExternalOutput")

    with tile.TileContext(nc) as tc:
        with (
            tc.tile_pool(name="const", bufs=1) as const,
            tc.tile_pool(name="bdp", bufs=NBD) as bdp,
            tc.tile_pool(name="dp", bufs=ND) as dp,
            tc.tile_pool(name="xp", bufs=3) as xp,
            tc.tile_pool(name="ap", bufs=3) as ap,
            tc.tile_pool(name="op", bufs=3) as op,
            tc.tile_pool(name="atp", bufs=3, space="PSUM") as atp,
            tc.tile_pool(name="yp", bufs=4, space="PSUM") as yp,
        ):
            ident_sb = const.tile([HW, HW], f32)
            nc.sync.dma_start(out=ident_sb, in_=id_d[:])
            alpha_sb = const.tile([128, 1], f32)
            nc.sync.dma_start(out=alpha_sb, in_=al_d[:])

            # rhs ring: zeros except the two alpha*A^T blocks written per pair
            bd_tiles = []
            for i in range(NBD):
                t = bdp.tile([128, 2, HW], f32, tag="bd")
                nc.vector.memset(t, 0.0)
                bd_tiles.append(t)

            # D-tile ring: gap rows 49:64 / 113:128 must stay finite (zero)
            d_tiles = []
            for i in range(ND):
                t = dp.tile([128, NPAIR, C], f32, tag="d")
                # zero the 32-aligned ranges covering the gap rows 49:64 and
                # 113:128; the DMA overwrites 32:49 / 96:113 with real data
                nc.vector.memset(t[32:64, :, :], 0.0)
                nc.vector.memset(t[96:128, :, :], 0.0)
                d_tiles.append(t)

            for g in range(NGROUP):
                b0 = g * G
                xs = x_d[b0:b0 + G]      # [G, C, HW]
                os_ = o_d[b0:b0 + G]
                ds = d_d[b0:b0 + G]      # [G, HW, C]
                as_ = a_d[b0:b0 + G]     # [G, HW, HW]

                x_t = xp.tile([128, G, 4 * HW], f32, tag="x")
                nc.sync.dma_start(
                    out=x_t, in_=xs.rearrange("n (r j) p -> r n (j p)", j=4)
                )
                d_t = d_tiles[g % ND]
                d_v = d_t.rearrange("(b s) i c -> b s i c", b=2)
                dsr = ds.rearrange("(i b) q c -> b q i c", b=2)
                # two plain partition-range DMAs (bases 0 and 64); they run
                # concurrently on complementary DMA-engine halves
                nc.sync.dma_start(out=d_v[0, 0:HW, :, :], in_=dsr[0])
                nc.sync.dma_start(out=d_v[1, 0:HW, :, :], in_=dsr[1])
                # attn in 64-wide slots so the pair transpose lands the odd
                # batch at PSUM rows 64:113
                a_t = ap.tile([HW, G, 64], f32, tag="a")
                nc.sync.dma_start(
                    out=a_t[:, :, 0:HW], in_=as_.rearrange("n p q -> p n q")
                )

                o_t = op.tile([128, G, 4 * HW], f32, tag="o")

                # views
                d4 = d_t.rearrange("k i (m four) -> k i four m", four=4)
                x4 = x_t.rearrange("r n (j p) -> r n j p", j=4)
                o4 = o_t.rearrange("r n (j p) -> r n j p", j=4)
                a2 = a_t.rearrange("p n q -> p (n q)")

                for i in range(NPAIR):
                    at_ps = atp.tile([128, HW], f32, tag="at")
                    # [49, 128] -> [128, 49]: rows b*64+q = A^T pair
                    nc.tensor.transpose(
                        at_ps, a2[:, 2 * i * 64:(2 * i + 2) * 64], ident_sb
                    )
                    bd = bd_tiles[i % NBD]
                    nc.vector.tensor_scalar_mul(
                        out=bd[0:HW, 0, :],
                        in0=at_ps[0:HW, :],
                        scalar1=alpha_sb[0:HW, :],
                    )
                    nc.vector.tensor_scalar_mul(
                        out=bd[64:64 + HW, 1, :],
                        in0=at_ps[64:64 + HW, :],
                        scalar1=alpha_sb[64:64 + HW, :],
                    )

                    y_ps = yp.tile([128, 4, 2 * HW], f32, tag="y")
                    bd2 = bd.rearrange("k b p -> k (b p)")
                    for j in range(4):
                        nc.tensor.matmul(
                            out=y_ps[:, j, :],
                            lhsT=d4[0:64 + HW, i, j, :],
                            rhs=bd2[0:64 + HW, :],
                            start=True,
                            stop=True,
                        )
                    # y_ps free layout: (j, b, p); regroup to (b, j, p)
                    y4 = y_ps.rearrange("r j (b p) -> r b j p", b=2)
                    nc.vector.tensor_add(
                        out=o4[:, 2 * i:2 * i + 2, :, :],
                        in0=y4,
                        in1=x4[:, 2 * i:2 * i + 2, :, :],
                    )

                nc.scalar.dma_start(
                    out=os_.rearrange("n (r j) p -> r n (j p)", j=4), in_=o_t
                )

    nc.finalize()
    return nc


def _get_nc():
    if "nc" not in _cached:
        _cached["nc"] = _build_bass()
    return _cached["nc"]


def _in_maps(x, attn, D, alpha):
    x_s = np.ascontiguousarray(x, dtype=np.float32).reshape(N_CORES, NB, C, HW)
    a_s = np.ascontiguousarray(attn, dtype=np.float32).reshape(N_CORES, NB, HW, HW)
    d_s = np.ascontiguousarray(D, dtype=np.float32).reshape(N_CORES, NB, HW, C)
    al = np.full((128, 1), np.float32(np.asarray(alpha).reshape(-1)[0]), np.float32)
    ident = np.eye(HW, dtype=np.float32)
    return [
        {"x": x_s[c], "attn": a_s[c], "d": d_s[c], "alphac": al, "ident": ident}
        for c in range(N_CORES)
    ]


def kernel(x: np.ndarray, attn: np.ndarray, D: np.ndarray, alpha: np.ndarray) -> np.ndarray:
    from concourse import bass_utils

    nc = _get_nc()
    res = bass_utils.run_bass_kernel_spmd(
        nc, _in_maps(x, attn, D, alpha), core_ids=list(range(N_CORES))
    )
    out = np.stack([res.results[c]["out"] for c in range(N_CORES)])
    return out.reshape(N_TOT, C, 7, 7).astype(np.float32, copy=False)



# revision 2
# speedup vs baseline: 1.9137x; 1.9137x over previous
"""Fused bmm + residual kernel for Trainium2 (8 NeuronCores, data-parallel).

out[n,c,p] = x[n,c,p] + alpha * sum_q attn[n,p,q] * D[n,q,c]
  N=2048, C=512, H=W=7 (HW=49)

Sharding: batch N across 8 cores (256 each), no collectives.

bf16 scheme (tolerance is 2e-2; bf16 keeps max rel err ~3e-3):
 - Host pre-packs all tensors to bf16 in the exact SBUF layouts, so every
   DMA is a contiguous partition-major block with multi-KB descriptors.
 - attn is transposed on the host (and alpha folded in), so the device
   does no transposes at all: per batch-pair, 4 matmuls + 1 add.
 - Pair packing with K=98, gap-free: even batch at partitions 0:49,
   odd at 49:98.  rhs [98, 98] is block-diagonal (off-diag zeroed once
   per ring tile); lhsT = D pair [98, 512] in c-chunks of 128.
 - PSUM y [128, 4, 98] fp32; DVE adds x and casts to bf16 for the store.
 - Load/store DMAs alternate between the two HWDGE rings per group.
"""
import sys

sys.path.insert(0, "/opt/trn_rl_repo")

import numpy as np
import ml_dtypes

BF16 = ml_dtypes.bfloat16

# ---- static problem config (hardcoded per harness contract) ----
N_TOT, C, HW = 2048, 512, 49
N_CORES = 8
NB = N_TOT // N_CORES        # 256 batches per core
G = 32                       # batches per group (one DMA round)
NPAIR = G // 2               # pairs per group
NGROUP = NB // G             # groups per core
NAT = 3                      # rhs (attn^T) ring size

_cached = {}


def _build_bass():
    import concourse.bacc as bacc
    import concourse.mybir as mybir
    from concourse import tile

    bf16 = mybir.dt.bfloat16
    f32 = mybir.dt.float32
    nc = bacc.Bacc(None, target_bir_lowering=False)

    x_d = nc.dram_tensor("xp", [NGROUP, 128, G, 4, HW], bf16, kind="ExternalInput")
    ae_d = nc.dram_tensor("ae", [NGROUP, HW, NPAIR, HW], bf16, kind="ExternalInput")
    ao_d = nc.dram_tensor("ao", [NGROUP, HW, NPAIR, HW], bf16, kind="ExternalInput")
    de_d = nc.dram_tensor("de", [NGROUP, HW, NPAIR, C], bf16, kind="ExternalInput")
    do_d = nc.dram_tensor("do", [NGROUP, HW, NPAIR, C], bf16, kind="ExternalInput")
    o_d = nc.dram_tensor("op", [NGROUP, 128, G, 4, HW], bf16, kind="ExternalOutput")

    with tile.TileContext(nc) as tc:
        with (
            tc.tile_pool(name="atp", bufs=NAT) as atp,
            tc.tile_pool(name="dp", bufs=3) as dp,
            tc.tile_pool(name="xp", bufs=3) as xp,
            tc.tile_pool(name="op", bufs=3) as op,
            tc.tile_pool(name="yp", bufs=8, space="PSUM") as yp,
        ):
            # rhs ring: block-diagonal [98, NPAIR, 98]; off-diagonal blocks
            # must stay zero, so memset each ring tile once up front.
            at_tiles = []
            for i in range(NAT):
                t = atp.tile([2 * HW, NPAIR, 2 * HW], bf16, tag="at")
                nc.vector.memset(t, 0.0)
                at_tiles.append(t)

            for g in range(NGROUP):
                ld = nc.sync if g % 2 == 0 else nc.scalar
                st = nc.scalar if g % 2 == 0 else nc.sync

                x_t = xp.tile([128, G, 4, HW], bf16, tag="x")
                ld.dma_start(out=x_t, in_=x_d[g])

                d_t = dp.tile([2 * HW, NPAIR, C], bf16, tag="d")
                ld.dma_start(out=d_t[0:HW], in_=de_d[g])
                ld.dma_start(out=d_t[HW:2 * HW], in_=do_d[g])

                at_t = at_tiles[g % NAT]
                ld.dma_start(out=at_t[0:HW, :, 0:HW], in_=ae_d[g])
                ld.dma_start(out=at_t[HW:2 * HW, :, HW:2 * HW], in_=ao_d[g])

                o_t = op.tile([128, G, 4, HW], bf16, tag="o")

                for i in range(NPAIR):
                    y_ps = yp.tile([128, 4, 2 * HW], f32, tag="y")
                    for j in range(4):
                        nc.tensor.matmul(
                            out=y_ps[:, j, :],
                            lhsT=d_t[0:2 * HW, i, 128 * j:128 * (j + 1)],
                            rhs=at_t[0:2 * HW, i, :],
                            start=True,
                            stop=True,
                        )
                    # y_ps free layout (j, b, p); regroup to (b, j, p)
                    y4 = y_ps.rearrange("r j (b p) -> r b j p", b=2)
                    nc.vector.tensor_add(
                        out=o_t[:, 2 * i:2 * i + 2, :, :],
                        in0=y4,
                        in1=x_t[:, 2 * i:2 * i + 2, :, :],
                    )

                st.dma_start(out=o_d[g], in_=o_t)

    nc.finalize()
    return nc


def _get_nc():
    if "nc" not in _cached:
        _cached["nc"] = _build_bass()
    return _cached["nc"]


def _in_maps(x, attn, D, alpha):
    a0 = np.float32(np.asarray(alpha).reshape(-1)[0])

    # x[n, c, p] -> xp[core, g, r, n, j, p] with c = 128j + r
    xr = np.asarray(x, dtype=np.float32).reshape(N_CORES, NGROUP, G, 4, 128, HW)
    xp = np.ascontiguousarray(xr.transpose(0, 1, 4, 2, 3, 5)).astype(BF16)

    # attn[n, p, q] * alpha -> attn_T[n, q, p] -> [core, g, q, i, p] even/odd
    at = (np.asarray(attn, dtype=np.float32) * a0).transpose(0, 2, 1)
    at = at.reshape(N_CORES, NGROUP, NPAIR, 2, HW, HW)
    ae = np.ascontiguousarray(at[:, :, :, 0].transpose(0, 1, 3, 2, 4)).astype(BF16)
    ao = np.ascontiguousarray(at[:, :, :, 1].transpose(0, 1, 3, 2, 4)).astype(BF16)

    # D[n, q, c] -> [core, g, q, i, c] even/odd
    dr = np.asarray(D, dtype=np.float32).reshape(N_CORES, NGROUP, NPAIR, 2, HW, C)
    de = np.ascontiguousarray(dr[:, :, :, 0].transpose(0, 1, 3, 2, 4)).astype(BF16)
    do = np.ascontiguousarray(dr[:, :, :, 1].transpose(0, 1, 3, 2, 4)).astype(BF16)

    return [
        {"xp": xp[c], "ae": ae[c], "ao": ao[c], "de": de[c], "do": do[c]}
        for c in range(N_CORES)
    ]


def kernel(x: np.ndarray, attn: np.ndarray, D: np.ndarray, alpha: np.ndarray) -> np.ndarray:
    from concourse import bass_utils

    nc = _get_nc()
    res = bass_utils.run_bass_kernel_spmd(
        nc, _in_maps(x, attn, D, alpha), core_ids=list(range(N_CORES))
    )
    # op[g, r, n, j, p] -> out[n, c, p] with c = 128j + r
    out = np.stack([res.results[c]["op"] for c in range(N_CORES)])
    out = out.astype(np.float32).transpose(0, 1, 3, 4, 2, 5)
    return np.ascontiguousarray(out).reshape(N_TOT, C, 7, 7)


# revision 6
# speedup vs baseline: 2.5474x; 1.3311x over previous
"""Fused bmm + residual kernel for Trainium2 (8 NeuronCores, data-parallel).

out[n,c,p] = x[n,c,p] + alpha * sum_q attn[n,p,q] * D[n,q,c]
  N=2048, C=512, H=W=7 (HW=49)

Sharding: batch N across 8 cores (256 each), no collectives.

bf16 scheme (tolerance is 2e-2; bf16 keeps max rel err ~4e-3):
 - Host pre-packs all tensors to bf16 in the exact SBUF layouts, so every
   DMA is a contiguous partition-major block with multi-KB descriptors.
 - attn is transposed on the host (and alpha folded in), so the device
   does no transposes at all: per batch-pair, 4 matmuls + 1 add.
 - Pair packing with K=98, gap-free: even batch at partitions 0:49,
   odd at 49:98.  rhs [98, (2, 49)] is block-diagonal (off-diag zeroed
   once per ring tile); lhsT = D pair [98, 512] in c-chunks of 128.
 - D loads as ONE 98-partition DMA per group (HWDGE assigns SDMA engines
   by relative-partition//8, so two 49-partition DMAs would pile onto
   engines 0-6; one 98-row DMA spreads over engines 0-12).
 - attn loads go through gpsimd (SWDGE) whose engine map differs.
 - PSUM y [128, 4, 98] fp32; residual adds alternate between Vector and
   GpSimd and cast to bf16 for the store.
 - Load/store DMAs alternate between the two HWDGE rings per group.
"""
import sys

sys.path.insert(0, "/opt/trn_rl_repo")

import numpy as np
import ml_dtypes

BF16 = ml_dtypes.bfloat16

# ---- static problem config (hardcoded per harness contract) ----
N_TOT, C, HW = 2048, 512, 49
N_CORES = 8
NB = N_TOT // N_CORES        # 256 batches per core
G = 32                       # batches per group (one DMA round)
NPAIR = G // 2               # pairs per group
NGROUP = NB // G             # groups per core
NAT = 3                      # rhs (attn^T) ring size

_cached = {}


def _build_bass():
    import concourse.bacc as bacc
    import concourse.mybir as mybir
    from concourse import tile

    bf16 = mybir.dt.bfloat16
    f32 = mybir.dt.float32
    nc = bacc.Bacc(None, target_bir_lowering=False)

    x_d = nc.dram_tensor("xp", [NGROUP, 128, G, 4, HW], bf16, kind="ExternalInput")
    ae_d = nc.dram_tensor("ae", [NGROUP, HW, NPAIR, HW], bf16, kind="ExternalInput")
    ao_d = nc.dram_tensor("ao", [NGROUP, HW, NPAIR, HW], bf16, kind="ExternalInput")
    d_d = nc.dram_tensor("dp", [NGROUP, 2 * HW, NPAIR, C], bf16, kind="ExternalInput")
    o_d = nc.dram_tensor("op", [NGROUP, 128, G, 4, HW], bf16, kind="ExternalOutput")

    with tile.TileContext(nc) as tc:
        with (
            tc.tile_pool(name="atp", bufs=NAT) as atp,
            tc.tile_pool(name="dp", bufs=3) as dp,
            tc.tile_pool(name="xp", bufs=3) as xp,
            tc.tile_pool(name="op", bufs=3) as op,
            tc.tile_pool(name="yp", bufs=8, space="PSUM") as yp,
        ):
            # rhs ring: block-diagonal [98, (2, 49)] per pair; off-diagonal
            # blocks must stay zero, so memset each ring tile once up front.
            at_tiles = []
            for i in range(NAT):
                t = atp.tile([2 * HW, 2, NPAIR, HW], bf16, tag="at")
                nc.vector.memset(t, 0.0)
                at_tiles.append(t)

            for g in range(NGROUP):
                ld = nc.sync if g % 2 == 0 else nc.scalar
                st = nc.scalar if g % 2 == 0 else nc.sync

                x_t = xp.tile([128, G, 4, HW], bf16, tag="x")
                ld.dma_start(out=x_t, in_=x_d[g])

                d_t = dp.tile([2 * HW, NPAIR, C], bf16, tag="d")
                ld.dma_start(out=d_t, in_=d_d[g])

                at_t = at_tiles[g % NAT]
                ld.dma_start(out=at_t[0:HW, 0, :, :], in_=ae_d[g])
                ld.dma_start(out=at_t[HW:2 * HW, 1, :, :], in_=ao_d[g])

                o_t = op.tile([128, G, 4, HW], bf16, tag="o")

                for i in range(NPAIR):
                    y_ps = yp.tile([128, 4, 2 * HW], f32, tag="y")
                    for j in range(4):
                        nc.tensor.matmul(
                            out=y_ps[:, j, :],
                            lhsT=d_t[0:2 * HW, i, 128 * j:128 * (j + 1)],
                            rhs=at_t[0:2 * HW, :, i, :],
                            start=True,
                            stop=True,
                        )
                    # y_ps free layout (j, b, p); regroup to (b, j, p)
                    y4 = y_ps.rearrange("r j (b p) -> r b j p", b=2)
                    nc.vector.tensor_add(
                        out=o_t[:, 2 * i:2 * i + 2, :, :],
                        in0=y4,
                        in1=x_t[:, 2 * i:2 * i + 2, :, :],
                    )

                st.dma_start(out=o_d[g], in_=o_t)

    nc.finalize()
    return nc


def _get_nc():
    if "nc" not in _cached:
        _cached["nc"] = _build_bass()
    return _cached["nc"]


def _in_maps(x, attn, D, alpha):
    a0 = np.float32(np.asarray(alpha).reshape(-1)[0])

    # x[n, c, p] -> xp[core, g, r, n, j, p] with c = 128j + r
    xr = np.asarray(x, dtype=np.float32).reshape(N_CORES, NGROUP, G, 4, 128, HW)
    xp = np.ascontiguousarray(xr.transpose(0, 1, 4, 2, 3, 5)).astype(BF16)

    # attn[n, p, q] * alpha -> attn_T[n, q, p] -> [core, g, q, i, p] even/odd
    at = (np.asarray(attn, dtype=np.float32) * a0).transpose(0, 2, 1)
    at = at.reshape(N_CORES, NGROUP, NPAIR, 2, HW, HW)
    ae = np.ascontiguousarray(at[:, :, :, 0].transpose(0, 1, 3, 2, 4)).astype(BF16)
    ao = np.ascontiguousarray(at[:, :, :, 1].transpose(0, 1, 3, 2, 4)).astype(BF16)

    # D[n, q, c] -> [core, g, b*49+q, i, c] (even rows 0:49, odd rows 49:98)
    dr = np.asarray(D, dtype=np.float32).reshape(N_CORES, NGROUP, NPAIR, 2, HW, C)
    dp = np.ascontiguousarray(dr.transpose(0, 1, 3, 4, 2, 5)).astype(BF16)
    dp = dp.reshape(N_CORES, NGROUP, 2 * HW, NPAIR, C)

    return [
        {"xp": xp[c], "ae": ae[c], "ao": ao[c], "dp": dp[c]}
        for c in range(N_CORES)
    ]


def kernel(x: np.ndarray, attn: np.ndarray, D: np.ndarray, alpha: np.ndarray) -> np.ndarray:
    from concourse import bass_utils

    nc = _get_nc()
    res = bass_utils.run_bass_kernel_spmd(
        nc, _in_maps(x, attn, D, alpha), core_ids=list(range(N_CORES))
    )
    # op[g, r, n, j, p] -> out[n, c, p] with c = 128j + r
    out = np.stack([res.results[c]["op"] for c in range(N_CORES)])
    out = out.astype(np.float32).transpose(0, 1, 3, 4, 2, 5)
    return np.ascontiguousarray(out).reshape(N_TOT, C, 7, 7)


# revision 7
# speedup vs baseline: 2.6054x; 1.0228x over previous
"""Fused bmm + residual kernel for Trainium2 (8 NeuronCores, data-parallel).

out[n,c,p] = x[n,c,p] + alpha * sum_q attn[n,p,q] * D[n,q,c]
  N=2048, C=512, H=W=7 (HW=49)

Sharding: batch N across 8 cores (256 each), no collectives.

bf16 scheme (tolerance is 2e-2; bf16 keeps max rel err ~4e-3):
 - Host pre-packs all tensors to bf16 in the exact SBUF layouts, so every
   DMA is a contiguous partition-major block with multi-KB descriptors.
 - attn is transposed on the host (and alpha folded in), so the device
   does no transposes at all: per batch-pair, 4 matmuls + 1 add.
 - Pair packing with K=98, gap-free: even batch at partitions 0:49,
   odd at 49:98.  rhs [98, (2, 49)] is block-diagonal (off-diag zeroed
   once per ring tile); lhsT = D pair [98, 512] in c-chunks of 128.
 - D loads as ONE 98-partition DMA per group (HWDGE assigns SDMA engines
   by relative-partition//8, so two 49-partition DMAs would pile onto
   engines 0-6; one 98-row DMA spreads over engines 0-12).
 - attn loads go through gpsimd (SWDGE) whose engine map differs.
 - PSUM y [128, 4, 98] fp32; residual adds alternate between Vector and
   GpSimd and cast to bf16 for the store.
 - Load/store DMAs alternate between the two HWDGE rings per group.
"""
import sys

sys.path.insert(0, "/opt/trn_rl_repo")

import numpy as np
import ml_dtypes

BF16 = ml_dtypes.bfloat16

# ---- static problem config (hardcoded per harness contract) ----
N_TOT, C, HW = 2048, 512, 49
N_CORES = 8
NB = N_TOT // N_CORES        # 256 batches per core
G = 16                       # batches per group (one DMA round)
NPAIR = G // 2               # pairs per group
NGROUP = NB // G             # groups per core
NAT = 4                      # rhs (attn^T) ring size

_cached = {}


def _build_bass():
    import concourse.bacc as bacc
    import concourse.mybir as mybir
    from concourse import tile

    bf16 = mybir.dt.bfloat16
    f32 = mybir.dt.float32
    nc = bacc.Bacc(None, target_bir_lowering=False)

    x_d = nc.dram_tensor("xp", [NGROUP, 128, G, 4, HW], bf16, kind="ExternalInput")
    ae_d = nc.dram_tensor("ae", [NGROUP, HW, NPAIR, HW], bf16, kind="ExternalInput")
    ao_d = nc.dram_tensor("ao", [NGROUP, HW, NPAIR, HW], bf16, kind="ExternalInput")
    d_d = nc.dram_tensor("dp", [NGROUP, 2 * HW, NPAIR, C], bf16, kind="ExternalInput")
    o_d = nc.dram_tensor("op", [NGROUP, 128, G, 4, HW], bf16, kind="ExternalOutput")

    with tile.TileContext(nc) as tc:
        with (
            tc.tile_pool(name="atp", bufs=NAT) as atp,
            tc.tile_pool(name="dp", bufs=4) as dp,
            tc.tile_pool(name="xp", bufs=4) as xp,
            tc.tile_pool(name="op", bufs=4) as op,
            tc.tile_pool(name="yp", bufs=8, space="PSUM") as yp,
        ):
            # rhs ring: block-diagonal [98, (2, 49)] per pair; off-diagonal
            # blocks must stay zero, so memset each ring tile once up front.
            at_tiles = []
            for i in range(NAT):
                t = atp.tile([2 * HW, 2, NPAIR, HW], bf16, tag="at")
                nc.vector.memset(t, 0.0)
                at_tiles.append(t)

            for g in range(NGROUP):
                ld = nc.sync if g % 2 == 0 else nc.scalar
                st = nc.scalar if g % 2 == 0 else nc.sync

                x_t = xp.tile([128, G, 4, HW], bf16, tag="x")
                ld.dma_start(out=x_t, in_=x_d[g])

                d_t = dp.tile([2 * HW, NPAIR, C], bf16, tag="d")
                ld.dma_start(out=d_t, in_=d_d[g])

                at_t = at_tiles[g % NAT]
                ld.dma_start(out=at_t[0:HW, 0, :, :], in_=ae_d[g])
                ld.dma_start(out=at_t[HW:2 * HW, 1, :, :], in_=ao_d[g])

                o_t = op.tile([128, G, 4, HW], bf16, tag="o")

                for i in range(NPAIR):
                    y_ps = yp.tile([128, 4, 2 * HW], f32, tag="y")
                    for j in range(4):
                        nc.tensor.matmul(
                            out=y_ps[:, j, :],
                            lhsT=d_t[0:2 * HW, i, 128 * j:128 * (j + 1)],
                            rhs=at_t[0:2 * HW, :, i, :],
                            start=True,
                            stop=True,
                        )
                    # y_ps free layout (j, b, p); regroup to (b, j, p)
                    y4 = y_ps.rearrange("r j (b p) -> r b j p", b=2)
                    nc.vector.tensor_add(
                        out=o_t[:, 2 * i:2 * i + 2, :, :],
                        in0=y4,
                        in1=x_t[:, 2 * i:2 * i + 2, :, :],
                    )

                st.dma_start(out=o_d[g], in_=o_t)

    nc.finalize()
    return nc


def _get_nc():
    if "nc" not in _cached:
        _cached["nc"] = _build_bass()
    return _cached["nc"]


def _in_maps(x, attn, D, alpha):
    a0 = np.float32(np.asarray(alpha).reshape(-1)[0])

    # x[n, c, p] -> xp[core, g, r, n, j, p] with c = 128j + r
    xr = np.asarray(x, dtype=np.float32).reshape(N_CORES, NGROUP, G, 4, 128, HW)
    xp = np.ascontiguousarray(xr.transpose(0, 1, 4, 2, 3, 5)).astype(BF16)

    # attn[n, p, q] * alpha -> attn_T[n, q, p] -> [core, g, q, i, p] even/odd
    at = (np.asarray(attn, dtype=np.float32) * a0).transpose(0, 2, 1)
    at = at.reshape(N_CORES, NGROUP, NPAIR, 2, HW, HW)
    ae = np.ascontiguousarray(at[:, :, :, 0].transpose(0, 1, 3, 2, 4)).astype(BF16)
    ao = np.ascontiguousarray(at[:, :, :, 1].transpose(0, 1, 3, 2, 4)).astype(BF16)

    # D[n, q, c] -> [core, g, b*49+q, i, c] (even rows 0:49, odd rows 49:98)
    dr = np.asarray(D, dtype=np.float32).reshape(N_CORES, NGROUP, NPAIR, 2, HW, C)
    dp = np.ascontiguousarray(dr.transpose(0, 1, 3, 4, 2, 5)).astype(BF16)
    dp = dp.reshape(N_CORES, NGROUP, 2 * HW, NPAIR, C)

    return [
        {"xp": xp[c], "ae": ae[c], "ao": ao[c], "dp": dp[c]}
        for c in range(N_CORES)
    ]


def kernel(x: np.ndarray, attn: np.ndarray, D: np.ndarray, alpha: np.ndarray) -> np.ndarray:
    from concourse import bass_utils

    nc = _get_nc()
    res = bass_utils.run_bass_kernel_spmd(
        nc, _in_maps(x, attn, D, alpha), core_ids=list(range(N_CORES))
    )
    # op[g, r, n, j, p] -> out[n, c, p] with c = 128j + r
    out = np.stack([res.results[c]["op"] for c in range(N_CORES)])
    out = out.astype(np.float32).transpose(0, 1, 3, 4, 2, 5)
    return np.ascontiguousarray(out).reshape(N_TOT, C, 7, 7)


# revision 8
# speedup vs baseline: 2.6073x; 1.0007x over previous
"""Fused bmm + residual kernel for Trainium2 (8 NeuronCores, data-parallel).

out[n,c,p] = x[n,c,p] + alpha * sum_q attn[n,p,q] * D[n,q,c]
  N=2048, C=512, H=W=7 (HW=49)

Sharding: batch N across 8 cores (256 each), no collectives.

bf16 scheme (tolerance is 2e-2; bf16 keeps max rel err ~4e-3):
 - Host pre-packs all tensors to bf16 in the exact SBUF layouts, so every
   DMA is a contiguous partition-major block with multi-KB descriptors.
 - attn is transposed on the host (and alpha folded in), so the device
   does no transposes at all: per batch-pair, 4 matmuls + 1 add.
 - Pair packing with K=98, gap-free: even batch at partitions 0:49,
   odd at 49:98.  rhs [98, (2, 49)] is block-diagonal (off-diag zeroed
   once per ring tile); lhsT = D pair [98, 512] in c-chunks of 128.
 - D loads as ONE 98-partition DMA per group (HWDGE spreads a DMA's
   descriptors over ~ceil(rows/7) SDMA engines by relative row, so
   49-row DMAs pile onto 7 engines; 98 rows use 14).
 - PSUM y [128, 4, 98] fp32; DVE adds x and casts to bf16 for the store.
 - Load/store DMAs alternate between the two HWDGE rings per group.
 - Variable group sizes: small warmup groups so the first matmuls start
   ~15us earlier, small tail groups to shorten the drain.
"""
import sys

sys.path.insert(0, "/opt/trn_rl_repo")

import numpy as np
import ml_dtypes

BF16 = ml_dtypes.bfloat16

# ---- static problem config (hardcoded per harness contract) ----
N_TOT, C, HW = 2048, 512, 49
N_CORES = 8
NB = N_TOT // N_CORES        # 256 batches per core
NPAIRS = NB // 2             # 128 pairs per core
GMAX = 16                    # max batches per group
GROUPS = [4, 4, 8] + [16] * 14 + [8, 4, 4]   # batches per group, sum=256
assert sum(GROUPS) == NB
NAT = 4                      # rhs (attn^T) ring size

_cached = {}


def _build_bass():
    import concourse.bacc as bacc
    import concourse.mybir as mybir
    from concourse import tile

    bf16 = mybir.dt.bfloat16
    f32 = mybir.dt.float32
    nc = bacc.Bacc(None, target_bir_lowering=False)

    x_d = nc.dram_tensor("xp", [128, NB, 4, HW], bf16, kind="ExternalInput")
    ae_d = nc.dram_tensor("ae", [HW, NPAIRS, HW], bf16, kind="ExternalInput")
    ao_d = nc.dram_tensor("ao", [HW, NPAIRS, HW], bf16, kind="ExternalInput")
    d_d = nc.dram_tensor("dp", [2 * HW, NPAIRS, C], bf16, kind="ExternalInput")
    o_d = nc.dram_tensor("op", [128, NB, 4, HW], bf16, kind="ExternalOutput")

    with tile.TileContext(nc) as tc:
        with (
            tc.tile_pool(name="atp", bufs=NAT) as atp,
            tc.tile_pool(name="dp", bufs=4) as dp,
            tc.tile_pool(name="xp", bufs=4) as xp,
            tc.tile_pool(name="op", bufs=4) as op,
            tc.tile_pool(name="yp", bufs=8, space="PSUM") as yp,
        ):
            # rhs ring: block-diagonal [98, (2, 49)] per pair; off-diagonal
            # blocks must stay zero, so memset each ring tile once up front.
            at_tiles = []
            for i in range(NAT):
                t = atp.tile([2 * HW, 2, GMAX // 2, HW], bf16, tag="at")
                nc.vector.memset(t, 0.0)
                at_tiles.append(t)

            b0 = 0
            for g, gsz in enumerate(GROUPS):
                npair = gsz // 2
                i0 = b0 // 2
                ld = nc.sync if g % 2 == 0 else nc.scalar
                st = nc.scalar if g % 2 == 0 else nc.sync

                x_t = xp.tile([128, GMAX, 4, HW], bf16, tag="x")
                ld.dma_start(out=x_t[:, 0:gsz], in_=x_d[:, b0:b0 + gsz])

                d_t = dp.tile([2 * HW, GMAX // 2, C], bf16, tag="d")
                ld.dma_start(out=d_t[:, 0:npair], in_=d_d[:, i0:i0 + npair])

                at_t = at_tiles[g % NAT]
                ld.dma_start(out=at_t[0:HW, 0, 0:npair, :],
                             in_=ae_d[:, i0:i0 + npair])
                ld.dma_start(out=at_t[HW:2 * HW, 1, 0:npair, :],
                             in_=ao_d[:, i0:i0 + npair])

                o_t = op.tile([128, GMAX, 4, HW], bf16, tag="o")

                for i in range(npair):
                    y_ps = yp.tile([128, 4, 2 * HW], f32, tag="y")
                    for j in range(4):
                        nc.tensor.matmul(
                            out=y_ps[:, j, :],
                            lhsT=d_t[0:2 * HW, i, 128 * j:128 * (j + 1)],
                            rhs=at_t[0:2 * HW, :, i, :],
                            start=True,
                            stop=True,
                        )
                    # y_ps free layout (j, b, p); regroup to (b, j, p)
                    y4 = y_ps.rearrange("r j (b p) -> r b j p", b=2)
                    nc.vector.tensor_add(
                        out=o_t[:, 2 * i:2 * i + 2, :, :],
                        in0=y4,
                        in1=x_t[:, 2 * i:2 * i + 2, :, :],
                    )

                st.dma_start(out=o_d[:, b0:b0 + gsz], in_=o_t[:, 0:gsz])
                b0 += gsz

    nc.finalize()
    return nc


def _get_nc():
    if "nc" not in _cached:
        _cached["nc"] = _build_bass()
    return _cached["nc"]


def _in_maps(x, attn, D, alpha):
    a0 = np.float32(np.asarray(alpha).reshape(-1)[0])

    # x[n, c, p] -> xp[core, r, n, j, p] with c = 128j + r
    xr = np.asarray(x, dtype=np.float32).reshape(N_CORES, NB, 4, 128, HW)
    xp = np.ascontiguousarray(xr.transpose(0, 3, 1, 2, 4)).astype(BF16)

    # attn[n, p, q] * alpha -> attn_T[n, q, p] -> [core, q, i, p] even/odd
    at = (np.asarray(attn, dtype=np.float32) * a0).transpose(0, 2, 1)
    at = at.reshape(N_CORES, NPAIRS, 2, HW, HW)
    ae = np.ascontiguousarray(at[:, :, 0].transpose(0, 2, 1, 3)).astype(BF16)
    ao = np.ascontiguousarray(at[:, :, 1].transpose(0, 2, 1, 3)).astype(BF16)

    # D[n, q, c] -> [core, b*49+q, i, c] (even rows 0:49, odd rows 49:98)
    dr = np.asarray(D, dtype=np.float32).reshape(N_CORES, NPAIRS, 2, HW, C)
    dp = np.ascontiguousarray(dr.transpose(0, 2, 3, 1, 4)).astype(BF16)
    dp = dp.reshape(N_CORES, 2 * HW, NPAIRS, C)

    return [
        {"xp": xp[c], "ae": ae[c], "ao": ao[c], "dp": dp[c]}
        for c in range(N_CORES)
    ]


def kernel(x: np.ndarray, attn: np.ndarray, D: np.ndarray, alpha: np.ndarray) -> np.ndarray:
    from concourse import bass_utils

    nc = _get_nc()
    res = bass_utils.run_bass_kernel_spmd(
        nc, _in_maps(x, attn, D, alpha), core_ids=list(range(N_CORES))
    )
    # op[r, n, j, p] -> out[n, c, p] with c = 128j + r
    out = np.stack([res.results[c]["op"] for c in range(N_CORES)])
    out = out.astype(np.float32).transpose(0, 2, 3, 1, 4)
    return np.ascontiguousarray(out).reshape(N_TOT, C, 7, 7)


# revision 9
# speedup vs baseline: 3.0370x; 1.1648x over previous
"""Fused bmm + residual kernel for Trainium2 (8 NeuronCores, data-parallel).

out[n,c,p] = x[n,c,p] + alpha * sum_q attn[n,p,q] * D[n,q,c]
  N=2048, C=512, H=W=7 (HW=49)

Sharding: batch N across 8 cores (256 each), no collectives.

bf16 scheme (tolerance is 2e-2; bf16 keeps max rel err ~4e-3):
 - Host pre-packs all tensors to bf16 in the exact SBUF layouts, so every
   DMA is a contiguous partition-major block with multi-KB descriptors.
 - attn is transposed on the host (and alpha folded in), so the device
   does no transposes at all: per batch-pair, 4 matmuls + 1 add.
 - Pair packing with K=98, gap-free: even batch at partitions 0:49,
   odd at 49:98.  rhs [98, (2, 49)] is block-diagonal (off-diag zeroed
   once per ring tile); lhsT = D pair [98, 512] in c-chunks of 128.
 - D loads as ONE 98-partition DMA per group (HWDGE spreads a DMA's
   descriptors over ~ceil(rows/7) SDMA engines by relative row, so
   49-row DMAs pile onto 7 engines; 98 rows use 14).
 - PSUM y [128, 4, 98] fp32; DVE adds x and casts to bf16 for the store.
 - x is loaded in fp8-e4m3: its rounding error is additive (max ~0.17 abs
   vs the 0.51 abs tolerance budget) and does not pass through the
   contraction, so fp8 is safe for x (but NOT for D/attn).
 - Load/store DMAs alternate between the two HWDGE rings per group.
 - Variable group sizes: small warmup groups so the first matmuls start
   ~15us earlier, small tail groups to shorten the drain.
"""
import sys

sys.path.insert(0, "/opt/trn_rl_repo")

import numpy as np
import ml_dtypes

BF16 = ml_dtypes.bfloat16
FP8 = ml_dtypes.float8_e4m3fn

# ---- static problem config (hardcoded per harness contract) ----
N_TOT, C, HW = 2048, 512, 49
N_CORES = 8
NB = N_TOT // N_CORES        # 256 batches per core
NPAIRS = NB // 2             # 128 pairs per core
GMAX = 16                    # max batches per group
GROUPS = [4, 4, 8] + [16] * 14 + [8, 4, 4]   # batches per group, sum=256
assert sum(GROUPS) == NB
NAT = 4                      # rhs (attn^T) ring size

_cached = {}


def _build_bass():
    import concourse.bacc as bacc
    import concourse.mybir as mybir
    from concourse import tile

    bf16 = mybir.dt.bfloat16
    f32 = mybir.dt.float32
    nc = bacc.Bacc(None, target_bir_lowering=False)

    fp8 = mybir.dt.float8e4
    x_d = nc.dram_tensor("xp", [128, NB, 4, HW], fp8, kind="ExternalInput")
    ae_d = nc.dram_tensor("ae", [HW, NPAIRS, HW], bf16, kind="ExternalInput")
    ao_d = nc.dram_tensor("ao", [HW, NPAIRS, HW], bf16, kind="ExternalInput")
    d_d = nc.dram_tensor("dp", [2 * HW, NPAIRS, C], bf16, kind="ExternalInput")
    o_d = nc.dram_tensor("op", [128, NB, 4, HW], bf16, kind="ExternalOutput")

    with tile.TileContext(nc) as tc:
        with (
            tc.tile_pool(name="atp", bufs=NAT) as atp,
            tc.tile_pool(name="dp", bufs=4) as dp,
            tc.tile_pool(name="xp", bufs=4) as xp,
            tc.tile_pool(name="op", bufs=4) as op,
            tc.tile_pool(name="yp", bufs=8, space="PSUM") as yp,
        ):
            # rhs ring: block-diagonal [98, (2, 49)] per pair; off-diagonal
            # blocks must stay zero, so memset each ring tile once up front.
            at_tiles = []
            for i in range(NAT):
                t = atp.tile([2 * HW, 2, GMAX // 2, HW], bf16, tag="at")
                nc.vector.memset(t, 0.0)
                at_tiles.append(t)

            b0 = 0
            for g, gsz in enumerate(GROUPS):
                npair = gsz // 2
                i0 = b0 // 2
                ld = nc.sync if g % 2 == 0 else nc.scalar
                st = nc.scalar if g % 2 == 0 else nc.sync

                x_t = xp.tile([128, GMAX, 4, HW], fp8, tag="x")
                ld.dma_start(out=x_t[:, 0:gsz], in_=x_d[:, b0:b0 + gsz])

                d_t = dp.tile([2 * HW, GMAX // 2, C], bf16, tag="d")
                ld.dma_start(out=d_t[:, 0:npair], in_=d_d[:, i0:i0 + npair])

                at_t = at_tiles[g % NAT]
                ld.dma_start(out=at_t[0:HW, 0, 0:npair, :],
                             in_=ae_d[:, i0:i0 + npair])
                ld.dma_start(out=at_t[HW:2 * HW, 1, 0:npair, :],
                             in_=ao_d[:, i0:i0 + npair])

                o_t = op.tile([128, GMAX, 4, HW], bf16, tag="o")

                for i in range(npair):
                    y_ps = yp.tile([128, 4, 2 * HW], f32, tag="y")
                    for j in range(4):
                        nc.tensor.matmul(
                            out=y_ps[:, j, :],
                            lhsT=d_t[0:2 * HW, i, 128 * j:128 * (j + 1)],
                            rhs=at_t[0:2 * HW, :, i, :],
                            start=True,
                            stop=True,
                        )
                    # y_ps free layout (j, b, p); regroup to (b, j, p)
                    y4 = y_ps.rearrange("r j (b p) -> r b j p", b=2)
                    nc.vector.tensor_add(
                        out=o_t[:, 2 * i:2 * i + 2, :, :],
                        in0=y4,
                        in1=x_t[:, 2 * i:2 * i + 2, :, :],
                    )

                st.dma_start(out=o_d[:, b0:b0 + gsz], in_=o_t[:, 0:gsz])
                b0 += gsz

    nc.finalize()
    return nc


def _get_nc():
    if "nc" not in _cached:
        _cached["nc"] = _build_bass()
    return _cached["nc"]


def _in_maps(x, attn, D, alpha):
    a0 = np.float32(np.asarray(alpha).reshape(-1)[0])

    # x[n, c, p] -> xp[core, r, n, j, p] with c = 128j + r
    xr = np.asarray(x, dtype=np.float32).reshape(N_CORES, NB, 4, 128, HW)
    xp = np.ascontiguousarray(xr.transpose(0, 3, 1, 2, 4)).astype(FP8)

    # attn[n, p, q] * alpha -> attn_T[n, q, p] -> [core, q, i, p] even/odd
    at = (np.asarray(attn, dtype=np.float32) * a0).transpose(0, 2, 1)
    at = at.reshape(N_CORES, NPAIRS, 2, HW, HW)
    ae = np.ascontiguousarray(at[:, :, 0].transpose(0, 2, 1, 3)).astype(BF16)
    ao = np.ascontiguousarray(at[:, :, 1].transpose(0, 2, 1, 3)).astype(BF16)

    # D[n, q, c] -> [core, b*49+q, i, c] (even rows 0:49, odd rows 49:98)
    dr = np.asarray(D, dtype=np.float32).reshape(N_CORES, NPAIRS, 2, HW, C)
    dp = np.ascontiguousarray(dr.transpose(0, 2, 3, 1, 4)).astype(BF16)
    dp = dp.reshape(N_CORES, 2 * HW, NPAIRS, C)

    return [
        {"xp": xp[c], "ae": ae[c], "ao": ao[c], "dp": dp[c]}
        for c in range(N_CORES)
    ]


def kernel(x: np.ndarray, attn: np.ndarray, D: np.ndarray, alpha: np.ndarray) -> np.ndarray:
    from concourse import bass_utils

    nc = _get_nc()
    res = bass_utils.run_bass_kernel_spmd(
        nc, _in_maps(x, attn, D, alpha), core_ids=list(range(N_CORES))
    )
    # op[r, n, j, p] -> out[n, c, p] with c = 128j + r
    out = np.stack([res.results[c]["op"] for c in range(N_CORES)])
    out = out.astype(np.float32).transpose(0, 2, 3, 1, 4)
    return np.ascontiguousarray(out).reshape(N_TOT, C, 7, 7)


# revision 11
# speedup vs baseline: 3.1287x; 1.0302x over previous
"""Fused bmm + residual kernel for Trainium2 (8 NeuronCores, data-parallel).

out[n,c,p] = x[n,c,p] + alpha * sum_q attn[n,p,q] * D[n,q,c]
  N=2048, C=512, H=W=7 (HW=49)

Sharding: batch N across 8 cores (256 each), no collectives.

Scheme (tolerance is 2e-2; measured max rel err ~1.1e-2):
 - Host pre-packs all tensors to device layouts, so every DMA is a
   contiguous partition-major block with multi-KB descriptors.
 - attn is transposed on the host (alpha folded in); no device transposes.
 - Pair packing with K=98, gap-free: even batch at partitions 0:49, odd
   at 49:98 (matmul operand partition base MUST be 0 here: bases 32/64
   pass bass asserts but base-64 aborts on this runtime).
 - rhs [98, (2, 49)] is block-diagonal, off-diag zeroed once per ring
   tile; lhsT = D pair [98, 512] in c-chunks of 128; 4 MMs + 1 add /pair.
 - D loads as ONE 98-row DMA per group: HWDGE spreads a DMA over
   min(16, ceil(rows/7)) SDMA engines by relative row, so 98 rows ride
   14 engines (49-row DMAs would pile onto 7).  Per-engine ~21.5 GB/s is
   the roofline; busiest-engine bytes set the kernel time.
 - x loads in fp8-e4m3: its rounding error is additive (~0.28 abs vs the
   ~0.51 abs tolerance budget) and does not pass through the contraction,
   so fp8 is safe for x but NOT for D/attn.  D/attn/out use bf16.
 - PSUM y [128, 4, 2, (2, 49)] fp32 covers TWO pairs; one DVE add per 2
   pairs (FD=784 amortizes the ~120-cycle DVE op startup).
 - Load/store DMAs alternate between the two HWDGE rings per group.
 - Variable group sizes: small warmup groups start the matmuls early,
   small tail groups shorten the drain.
"""
import sys

sys.path.insert(0, "/opt/trn_rl_repo")

import numpy as np
import ml_dtypes

BF16 = ml_dtypes.bfloat16
FP8 = ml_dtypes.float8_e4m3fn

# ---- static problem config (hardcoded per harness contract) ----
N_TOT, C, HW = 2048, 512, 49
N_CORES = 8
NB = N_TOT // N_CORES        # 256 batches per core
NPAIRS = NB // 2             # 128 pairs per core
GMAX = 16                    # max batches per group
GROUPS = [4, 4, 8] + [16] * 14 + [8, 4, 4]   # batches per group, sum=256
assert sum(GROUPS) == NB and all(g % 4 == 0 for g in GROUPS)
NAT = 4                      # rhs (attn^T) ring size

_cached = {}


def _build_bass():
    import concourse.bacc as bacc
    import concourse.mybir as mybir
    from concourse import tile

    bf16 = mybir.dt.bfloat16
    fp8 = mybir.dt.float8e4
    f32 = mybir.dt.float32
    nc = bacc.Bacc(None, target_bir_lowering=False)

    x_d = nc.dram_tensor("xp", [128, NB, 4, HW], fp8, kind="ExternalInput")
    ae_d = nc.dram_tensor("ae", [HW, NPAIRS, HW], bf16, kind="ExternalInput")
    ao_d = nc.dram_tensor("ao", [HW, NPAIRS, HW], bf16, kind="ExternalInput")
    d_d = nc.dram_tensor("dp", [2 * HW, NPAIRS, C], bf16, kind="ExternalInput")
    o_d = nc.dram_tensor("op", [128, NB, 4, HW], bf16, kind="ExternalOutput")

    with tile.TileContext(nc) as tc:
        with (
            tc.tile_pool(name="atp", bufs=NAT) as atp,
            tc.tile_pool(name="dp", bufs=4) as dp,
            tc.tile_pool(name="xp", bufs=4) as xp,
            tc.tile_pool(name="op", bufs=4) as op,
            tc.tile_pool(name="yp", bufs=4, space="PSUM") as yp,
        ):
            # rhs ring: block-diagonal [98, (2, 49)] per pair; off-diagonal
            # blocks must stay zero, so memset each ring tile once up front.
            at_tiles = []
            for i in range(NAT):
                t = atp.tile([2 * HW, 2, GMAX // 2, HW], bf16, tag="at")
                nc.vector.memset(t, 0.0)
                at_tiles.append(t)

            b0 = 0
            for g, gsz in enumerate(GROUPS):
                npair = gsz // 2
                i0 = b0 // 2
                ld = nc.sync if g % 2 == 0 else nc.scalar
                st = nc.scalar if g % 2 == 0 else nc.sync

                x_t = xp.tile([128, GMAX, 4, HW], fp8, tag="x")
                ld.dma_start(out=x_t[:, 0:gsz], in_=x_d[:, b0:b0 + gsz])

                d_t = dp.tile([2 * HW, GMAX // 2, C], bf16, tag="d")
                ld.dma_start(out=d_t[:, 0:npair], in_=d_d[:, i0:i0 + npair])

                at_t = at_tiles[g % NAT]
                ld.dma_start(out=at_t[0:HW, 0, 0:npair, :],
                             in_=ae_d[:, i0:i0 + npair])
                ld.dma_start(out=at_t[HW:2 * HW, 1, 0:npair, :],
                             in_=ao_d[:, i0:i0 + npair])

                o_t = op.tile([128, GMAX, 4, HW], bf16, tag="o")

                for ii in range(npair // 2):
                    # y covers two pairs: free dims (j, u, (b, p))
                    y_ps = yp.tile([128, 4, 2, 2 * HW], f32, tag="y")
                    for u in range(2):
                        i = 2 * ii + u
                        for j in range(4):
                            nc.tensor.matmul(
                                out=y_ps[:, j, u, :],
                                lhsT=d_t[0:2 * HW, i, 128 * j:128 * (j + 1)],
                                rhs=at_t[0:2 * HW, :, i, :],
                                start=True,
                                stop=True,
                            )
                    # regroup (j, u, b, p) -> (u, b, j, p) = (n, j, p)
                    y4 = y_ps.rearrange("r j u (b p) -> r (u b) j p", b=2)
                    nc.vector.tensor_add(
                        out=o_t[:, 4 * ii:4 * ii + 4, :, :],
                        in0=y4,
                        in1=x_t[:, 4 * ii:4 * ii + 4, :, :],
                    )

                st.dma_start(out=o_d[:, b0:b0 + gsz], in_=o_t[:, 0:gsz])
                b0 += gsz

    nc.finalize()
    return nc


def _get_nc():
    if "nc" not in _cached:
        _cached["nc"] = _build_bass()
    return _cached["nc"]


def _in_maps(x, attn, D, alpha):
    a0 = np.float32(np.asarray(alpha).reshape(-1)[0])

    # x[n, c, p] -> xp[core, r, n, j, p] with c = 128j + r
    xr = np.asarray(x, dtype=np.float32).reshape(N_CORES, NB, 4, 128, HW)
    xp = np.ascontiguousarray(xr.transpose(0, 3, 1, 2, 4)).astype(FP8)

    # attn[n, p, q] * alpha -> attn_T[n, q, p] -> [core, q, i, p] even/odd
    at = (np.asarray(attn, dtype=np.float32) * a0).transpose(0, 2, 1)
    at = at.reshape(N_CORES, NPAIRS, 2, HW, HW)
    ae = np.ascontiguousarray(at[:, :, 0].transpose(0, 2, 1, 3)).astype(BF16)
    ao = np.ascontiguousarray(at[:, :, 1].transpose(0, 2, 1, 3)).astype(BF16)

    # D[n, q, c] -> [core, b*49+q, i, c] (even rows 0:49, odd rows 49:98)
    dr = np.asarray(D, dtype=np.float32).reshape(N_CORES, NPAIRS, 2, HW, C)
    dp = np.ascontiguousarray(dr.transpose(0, 2, 3, 1, 4)).astype(BF16)
    dp = dp.reshape(N_CORES, 2 * HW, NPAIRS, C)

    return [
        {"xp": xp[c], "ae": ae[c], "ao": ao[c], "dp": dp[c]}
        for c in range(N_CORES)
    ]


def kernel(x: np.ndarray, attn: np.ndarray, D: np.ndarray, alpha: np.ndarray) -> np.ndarray:
    from concourse import bass_utils

    nc = _get_nc()
    res = bass_utils.run_bass_kernel_spmd(
        nc, _in_maps(x, attn, D, alpha), core_ids=list(range(N_CORES))
    )
    # op[r, n, j, p] -> out[n, c, p] with c = 128j + r
    out = np.stack([res.results[c]["op"] for c in range(N_CORES)])
    out = out.astype(np.float32).transpose(0, 2, 3, 1, 4)
    return np.ascontiguousarray(out).reshape(N_TOT, C, 7, 7)
